# revision 1
# baseline (speedup 1.0000x reference)
"""LorentzBatchNorm2d Trainium2 kernel (8-core SPMD), v6.

Input x: [64, 64, 64, 64] (B, C, H, W) float32, gamma/beta: [63].
Sharded data-parallel over B (8 planes per core). Channels live on SBUF
partitions (top pixel-half on partitions 0-63, bottom half on 64-127);
pixels along the free axis.

v6 layout/structure:
- P2 alpha matmuls emit alpha rows at 2t/2t+1 (partitions 0..63) AND
  alpha+x0 rows at 64+2t/65+2t (partitions 64..127) from one accumulating
  PSUM bank, by giving each per-tile weight block four runtime columns
  (mu_signed masked top/bot, and the same plus e0/e64).
- The arccosh/coef chain runs once on [64, 512] with fused
  scalar_tensor_tensor ops and ACT Rsqrt (no DVE reciprocal).
- The whole P3 front (row DMAs, coef-broadcast and correction matmuls,
  fused DVE ops) is AR2-independent: invsd is applied late, per tile,
  as ACT Identity(scale=invsd, bias=beta) reading the pre-sum from the
  recycled cc PSUM bank. The coef-broadcast lhsT is a constant and the
  correction lhsT only needs AR1, so the P3 pipeline fills the AR2
  collective's latency window.
- P3 per tile: two rank-2 f32r matmuls (coef broadcast -> bank A,
  correction -> bank B), fused DVE e=(x*gamma)*A, DVE add pre=e+B back
  into bank A, ACT Identity(iv,beta) -> out_s, ACT Square into bf16,
  and a lagged bf16 tsum matmul accumulating per-pixel sum(s^2).
- P1 (channel sums) and AR1 for rep r+1 are emitted at the end of rep r
  and complete during the stores/mu/P2 of the next rep.
"""

import sys

sys.path.insert(0, "/opt/trn_rl_repo")

import numpy as np

import concourse.bass as bass
import concourse.tile as tile
from concourse import mybir

f32 = mybir.dt.float32
f32r = mybir.dt.float32r
bf16 = mybir.dt.bfloat16
AF = mybir.ActivationFunctionType
ALU = mybir.AluOpType

B, C, H, W = 64, 64, 64, 64
EPS = 1e-5
NCORES = 8
PPC = B // NCORES          # planes (b indices) per core = 8
HWP = H * W                # pixels per plane = 4096
PIX = PPC * HWP            # pixels per core = 32768
HALF = PIX // 2            # 16384 per partition-half
F = 512                    # pixels per tile (one PSUM bank of fp32)
NT = HALF // F             # 32 tiles per core
N_TOTAL = B * H * W        # 262144

DIRECT_RHS = False          # PE reads cf0/cr0 rows in place (no row DMAs)
TSUM_LAG = 4               # tiles of lag before emitting the tsum matmul
NP1 = 16                    # P1 chunks


def build_program(repeat: int = 1, phases: int = 3):
    nc = bass.Bass(num_devices=NCORES)

    x_d = nc.declare_dram_parameter("x", [PPC, C, HWP], f32, isOutput=False)
    out_d = nc.declare_dram_parameter("out", [PPC, C, HWP], f32, isOutput=True)

    sign_d = nc.declare_dram_parameter("sign_col", [128, 1], f32, isOutput=False)
    masktop_d = nc.declare_dram_parameter("masktop_col", [128, 1], f32, isOutput=False)
    maskbot_d = nc.declare_dram_parameter("maskbot_col", [128, 1], f32, isOutput=False)
    gneg_d = nc.declare_dram_parameter("gneg_col", [128, 1], f32, isOutput=False)
    gpos_d = nc.declare_dram_parameter("gpos_col", [128, 1], f32, isOutput=False)
    beta_d = nc.declare_dram_parameter("beta_col", [128, 1], f32, isOutput=False)
    e0t_d = nc.declare_dram_parameter("e0t_col", [128, 1], f32, isOutput=False)
    e0b_d = nc.declare_dram_parameter("e0b_col", [128, 1], f32, isOutput=False)
    onesrow_d = nc.declare_dram_parameter("ones_row", [1, 128], f32, isOutput=False)
    ident_d = nc.declare_dram_parameter("identity", [128, 128], f32, isOutput=False)
    fold_d = nc.declare_dram_parameter("fold128", [128, 128], f32, isOutput=False)
    ccrows_d = nc.declare_dram_parameter("cc_rows", [2, 128], f32, isOutput=False)
    albase_d = nc.declare_dram_parameter(
        "alhsT_base", [128, 128 * NT], f32, isOutput=False
    )
    tones_d = nc.declare_dram_parameter(
        "tones_all", [128, 128 * NT], bf16, isOutput=False
    )

    ar1_in = nc.dram_tensor("ar1_in", [128], f32)
    ar1_out = nc.dram_tensor("ar1_out", [128], f32, addr_space="Shared")
    ar2_in = nc.dram_tensor("ar2_in", [1], f32)
    ar2_out = nc.dram_tensor("ar2_out", [1], f32, addr_space="Shared")

    rg = [list(range(NCORES))]

    from contextlib import ExitStack

    with tile.TileContext(nc) as tc:
        with ExitStack() as stack:
            resident = stack.enter_context(tc.tile_pool(name="resident", bufs=1))
            singles = stack.enter_context(tc.tile_pool(name="singles", bufs=1))
            percol = stack.enter_context(tc.tile_pool(name="percol", bufs=2))
            work = stack.enter_context(tc.tile_pool(name="work", bufs=2))
            sqpool = stack.enter_context(tc.tile_pool(name="sqpool", bufs=6))
            psA = stack.enter_context(tc.tile_pool(name="psA", bufs=3, space="PSUM"))
            psB = stack.enter_context(tc.tile_pool(name="psB", bufs=2, space="PSUM"))
            psT = stack.enter_context(tc.tile_pool(name="psT", bufs=1, space="PSUM"))
            psS = stack.enter_context(tc.tile_pool(name="psS", bufs=1, space="PSUM"))
            psP = stack.enter_context(tc.tile_pool(name="psP", bufs=1, space="PSUM"))

            x_sb = resident.tile([128, HALF], f32)
            out_s = resident.tile([128, HALF], f32)
            alhsT = resident.tile([128, 128 * NT], f32)
            tones = resident.tile([128, 128 * NT], bf16)
            sink = resident.tile([128, HALF // NP1], f32)

            sign_c = singles.tile([128, 1], f32)
            masktop_c = singles.tile([128, 1], f32)
            maskbot_c = singles.tile([128, 1], f32)
            gneg_c = singles.tile([128, 1], f32)
            gpos_c = singles.tile([128, 1], f32)
            beta_c = singles.tile([128, 1], f32)
            e0t_c = singles.tile([128, 1], f32)
            e0b_c = singles.tile([128, 1], f32)
            onesrow_c = singles.tile([1, 128], f32)
            ident_c = singles.tile([128, 128], f32)
            fold_c = singles.tile([128, 128], f32)
            for dst, src in (
                (sign_c, sign_d), (masktop_c, masktop_d), (maskbot_c, maskbot_d),
                (gneg_c, gneg_d), (gpos_c, gpos_d), (beta_c, beta_d),
                (e0t_c, e0t_d), (e0b_c, e0b_d), (onesrow_c, onesrow_d),
                (ident_c, ident_d), (fold_c, fold_d),
            ):
                nc.sync.dma_start(out=dst[:], in_=src[:])
            nc.sync.dma_start(
                out=alhsT[:].bitcast(f32r), in_=albase_d[:].bitcast(f32r)
            )
            ccrows_c = singles.tile([2, 128], f32)
            nc.sync.dma_start(
                out=ccrows_c[:].bitcast(f32r), in_=ccrows_d[:].bitcast(f32r)
            )
            nc.sync.dma_start(out=tones[:], in_=tones_d[:])

            zero_c = singles.tile([128, 1], f32)
            nc.vector.memset(zero_c[:], 0.0)
            neg1_c = singles.tile([128, 1], f32)
            nc.vector.memset(neg1_c[:], -1.0)
            pone_c = singles.tile([128, 1], f32)
            nc.vector.memset(pone_c[:], 1.0)

            # PE primers: absorb const-DMA waits one semaphore at a time
            prime_ps = psP.tile([1, 16 + repeat], f32)
            prime_n = [0]

            def pe_prime(col_ap):
                j = prime_n[0]
                prime_n[0] += 1
                nc.tensor.matmul(
                    out=prime_ps[0:1, j:j + 1], lhsT=col_ap, rhs=col_ap
                )

            pe_prime(ident_c[0:128, 0:1])
            pe_prime(fold_c[0:128, 0:1])
            pe_prime(alhsT[0:128, 0:1])
            pe_prime(onesrow_c[0:1, 0:1])

            # ---- load x (4 paired-plane DMAs) ----
            for q in range(4):
                nc.sync.dma_start(
                    out=x_sb[:, q * HWP:(q + 1) * HWP].bitcast(f32r),
                    in_=x_d[q:PPC:4].bitcast(f32r),
                )
            pe_prime(x_sb[0:128, 0:1])
            pe_prime(tones[0:128, 0:2].bitcast(f32))

            CH = HALF // NP1  # P1 chunk width

            def emit_p1(r):
                """Channel sums for rep r: 8 ACT copy+accum chunks."""
                pcol = percol.tile([128, NP1], f32, tag="pcol")
                for cth in range(NP1):
                    nc.scalar.activation(
                        out=sink[:],
                        in_=x_sb[:, cth * CH:(cth + 1) * CH], func=AF.Copy,
                        accum_out=pcol[:, cth:cth + 1],
                    )
                ssum = percol.tile([128, 1], f32, tag="ssum")
                nc.vector.reduce_sum(
                    out=ssum[:], in_=pcol[:], axis=mybir.AxisListType.X
                )
                nc.sync.dma_start(out=ar1_in[:], in_=ssum[:])

            def emit_ar1(r):
                nc.gpsimd.collective_compute(
                    "AllReduce", ALU.add, replica_groups=rg,
                    ins=[ar1_in[:]], outs=[ar1_out[:]],
                )

            # prologue: P1 + AR1 for rep 0
            emit_p1(0)
            emit_ar1(0)

            psS_t = psS.tile([128, 128], f32)

            def small_mm(lhsT, rhs, shape, tag="small"):
                ps = psS_t[0:shape[0], 0:shape[1]]
                nc.tensor.matmul(out=ps, lhsT=lhsT, rhs=rhs)
                sb = singles.tile(shape, f32, tag=f"sb_{tag}")
                nc.scalar.copy(out=sb[:], in_=ps)
                return sb

            def pe_bcast(v11, tag):
                """Broadcast a [1,1] partition-0 scalar to a [128,1] col."""
                ps = psS_t[:, 0:1]
                nc.tensor.matmul(out=ps, lhsT=onesrow_c[:], rhs=v11[:])
                col = singles.tile([128, 1], f32, tag=f"bc_{tag}")
                nc.scalar.copy(out=col[:], in_=ps)
                return col

            for _rep in range(repeat):
                last = _rep == repeat - 1

                # ---- mu chain (needs AR1 of this rep) ----
                Sg = singles.tile([128, 1], f32, tag="Sg")
                nc.sync.dma_start(out=Sg[:], in_=ar1_out[:])
                Scol = small_mm(fold_c[:], Sg[:], [128, 1], tag="fold")
                ss = singles.tile([128, 1], f32, tag="ss")
                nc.vector.scalar_tensor_tensor(
                    out=ss[:], in0=Scol[:], scalar=sign_c[:], in1=Scol[:],
                    op0=ALU.mult, op1=ALU.mult,
                )
                qsb = small_mm(ss[:], pone_c[:], [1, 1], tag="qmm")
                usb = singles.tile([1, 1], f32, tag="usb")
                nc.vector.tensor_scalar(
                    out=usb[:], in0=qsb[:], scalar1=0.5,
                    scalar2=EPS * float(N_TOTAL) ** 2,
                    op0=ALU.mult, op1=ALU.max,
                )
                su11 = singles.tile([1, 1], f32, tag="su11")
                nc.scalar.activation(
                    out=su11[:], in_=usb[:], func=AF.Sqrt, bias=zero_c[0:1]
                )
                rs11 = singles.tile([1, 1], f32, tag="rs11")
                nc.vector.reciprocal(out=rs11[:], in_=su11[:])
                rs_col = pe_bcast(rs11, "rs")
                mu_col = singles.tile([128, 1], f32, tag="mu")
                nc.vector.tensor_mul(out=mu_col[:], in0=Scol[:], in1=rs_col[:])
                mus_col = singles.tile([128, 1], f32, tag="mus")
                nc.vector.tensor_mul(out=mus_col[:], in0=mu_col[:], in1=sign_c[:])
                # masked mu columns and alpha+x0 columns
                mut = singles.tile([128, 1], f32, tag="mut")
                nc.vector.tensor_mul(out=mut[:], in0=mus_col[:], in1=masktop_c[:])
                mub = singles.tile([128, 1], f32, tag="mub")
                nc.vector.tensor_mul(out=mub[:], in0=mus_col[:], in1=maskbot_c[:])
                aat = singles.tile([128, 1], f32, tag="aat")
                nc.vector.tensor_add(out=aat[:], in0=mut[:], in1=e0t_c[:])
                aab = singles.tile([128, 1], f32, tag="aab")
                nc.vector.tensor_add(out=aab[:], in0=mub[:], in1=e0b_c[:])
                # scatter into per-tile weight blocks: col(t) = 130t + off
                al_ap = alhsT[:]
                for off, src_c in ((0, mut), (1, mub), (64, aat), (65, aab)):
                    dst = bass.AP(
                        tensor=al_ap.tensor, offset=al_ap.offset + off,
                        ap=[[128 * NT, 128], [130, NT], [1, 1]],
                    )
                    sap = src_c[:]
                    srcb = bass.AP(
                        tensor=sap.tensor, offset=sap.offset,
                        ap=[[1, 128], [0, NT], [1, 1]],
                    )
                    nc.vector.tensor_copy(out=dst.bitcast(f32r), in_=srcb)
                # w0n = -gamma * mu_s * 1/(1+mu0)
                i11 = singles.tile([1, 1], f32, tag="i11")
                nc.vector.tensor_scalar_add(
                    out=i11[:], in0=mu_col[0:1, 0:1], scalar1=1.0
                )
                inv11 = singles.tile([1, 1], f32, tag="inv11")
                nc.vector.reciprocal(out=inv11[:], in_=i11[:])
                inv_col = pe_bcast(inv11, "inv")
                w0a = singles.tile([128, 1], f32, tag="w0a")
                nc.vector.tensor_mul(out=w0a[:], in0=gneg_c[:], in1=mu_col[:])
                w0n = singles.tile([128, 1], f32, tag="w0n")
                nc.vector.tensor_mul(out=w0n[:], in0=w0a[:], in1=inv_col[:])
                # w2col: w0n masked split; lhsCR is AR2-independent now
                w2col = singles.tile([128, 2], f32, tag="w2col")
                nc.vector.tensor_mul(
                    out=w2col[:, 0:1], in0=w0n[:], in1=masktop_c[:]
                )
                nc.vector.tensor_mul(
                    out=w2col[:, 1:2], in0=w0n[:], in1=maskbot_c[:]
                )
                wcr_ps = psS_t[0:2, :]
                nc.tensor.transpose(
                    out=wcr_ps, in_=w2col[:], identity=ident_c[:]
                )
                lhsCR = singles.tile([2, 128], f32, tag="lhsCR")
                nc.scalar.copy(out=lhsCR[:].bitcast(f32r), in_=wcr_ps)

                # ---- P2: alpha / alpha+x0 matmuls into one PSUM bank ----
                apsum = psT.tile([128, F], f32, tag="acc")
                for t in range(NT):
                    nc.tensor.matmul(
                        out=apsum[:],
                        lhsT=alhsT[:, 128 * t:128 * (t + 1)].bitcast(f32r),
                        rhs=x_sb[:, t * F:(t + 1) * F].bitcast(f32r),
                        start=(t == 0), stop=(t == NT - 1),
                        skip_group_check=True,
                    )
                abank = singles.tile([128, F], f32, tag="abank")
                nc.scalar.copy(out=abank[:], in_=apsum[:])

                # ---- batched per-pixel chain on [64, F] ----
                aash = singles.tile([64, F], f32, tag="aash")
                nc.sync.dma_start(out=aash[:], in_=abank[64:128, :])
                cb = singles.tile([64, F], f32, tag="cb")
                nc.vector.tensor_scalar_max(
                    out=cb[:], in0=abank[0:64, :], scalar1=1.0 + EPS
                )
                q2 = singles.tile([64, F], f32, tag="q2")
                nc.scalar.activation(
                    out=q2[:], in_=cb[:], func=AF.Square, bias=zero_c[0:64]
                )
                sq = singles.tile([64, F], f32, tag="sq")
                nc.scalar.activation(
                    out=sq[:], in_=q2[:], func=AF.Sqrt, bias=neg1_c[0:64]
                )
                t1 = singles.tile([64, F], f32, tag="t1")
                nc.vector.tensor_add(out=t1[:], in0=cb[:], in1=sq[:])
                dd = singles.tile([64, F], f32, tag="dd")
                nc.scalar.activation(
                    out=dd[:], in_=t1[:], func=AF.Ln, bias=zero_c[0:64]
                )
                d2col = singles.tile([64, 1], f32, tag="d2col")
                d2junk = singles.tile([64, F], f32, tag="d2junk")
                nc.vector.scalar_tensor_tensor(
                    out=d2junk[:], in0=dd[:], scalar=1.0, in1=dd[:],
                    op0=ALU.mult, op1=ALU.mult, accum_out=d2col[:],
                )
                dsq_sb = small_mm(d2col[:], pone_c[0:64, :], [1, 1], tag="dsq")
                rsq = singles.tile([64, F], f32, tag="rsq")
                nc.vector.reciprocal(out=rsq[:], in_=sq[:])
                cf0 = singles.tile([64, F], f32, tag="cf0")
                nc.vector.tensor_tensor(
                    out=cf0[:].bitcast(f32r), in0=dd[:], in1=rsq[:],
                    op=ALU.mult,
                )
                cr0 = singles.tile([64, F], f32, tag="cr0")
                nc.vector.tensor_tensor(
                    out=cr0[:].bitcast(f32r), in0=cf0[:], in1=aash[:],
                    op=ALU.mult,
                )

                # ---- AR2 launch ----
                nc.sync.dma_start(out=ar2_in[:], in_=dsq_sb[:])
                nc.gpsimd.collective_compute(
                    "AllReduce", ALU.add, replica_groups=rg,
                    ins=[ar2_in[:]], outs=[ar2_out[:]],
                )



                # ---- AR2 result -> invsd -> lhs4 ----
                vg = singles.tile([1, 1], f32, tag="vg")
                nc.sync.dma_start(out=vg[:], in_=ar2_out[:])
                sv11 = singles.tile([1, 1], f32, tag="sv11")
                nc.scalar.activation(
                    out=sv11[:], in_=vg[:], func=AF.Sqrt, bias=zero_c[0:1],
                    scale=1.0 / float(N_TOTAL),
                )
                nc.vector.tensor_scalar_add(out=sv11[:], in0=sv11[:], scalar1=EPS)
                iv11 = singles.tile([1, 1], f32, tag="iv11")
                nc.vector.reciprocal(out=iv11[:], in_=sv11[:])
                iv_col = pe_bcast(iv11, "iv")

                # ---- P3 ----
                tpsum = psT.tile([128, F], f32, tag="acc")
                sq_tiles = [None] * NT

                def emit_tsum(j):
                    nc.tensor.matmul(
                        out=tpsum[:],
                        lhsT=tones[:, 128 * j:128 * (j + 1)],
                        rhs=sq_tiles[j][:],
                        start=(j == 0), stop=(j == NT - 1),
                        skip_group_check=True,
                    )

                for t in range(NT):
                    xs = x_sb[:, t * F:(t + 1) * F]
                    cf2c = work.tile([2, F], f32, tag="cf2c")
                    cr2c = work.tile([2, F], f32, tag="cr2c")
                    nc.gpsimd.dma_start(
                        out=cf2c[:].bitcast(f32r),
                        in_=cf0[2 * t:2 * t + 2, :].bitcast(f32r),
                    )
                    nc.gpsimd.dma_start(
                        out=cr2c[:].bitcast(f32r),
                        in_=cr0[2 * t:2 * t + 2, :].bitcast(f32r),
                    )
                    cc_ps = psA.tile([128, F], f32, tag="cc")
                    nc.tensor.matmul(
                        out=cc_ps[:], lhsT=ccrows_c[:].bitcast(f32r),
                        rhs=cf2c[:].bitcast(f32r),
                    )
                    corr_ps = psB.tile([128, F], f32, tag="corr")
                    nc.tensor.matmul(
                        out=corr_ps[:], lhsT=lhsCR[:].bitcast(f32r),
                        rhs=cr2c[:].bitcast(f32r),
                    )
                    e_t = work.tile([128, F], f32, tag="e_t")
                    nc.vector.scalar_tensor_tensor(
                        out=e_t[:], in0=xs, scalar=gpos_c[:], in1=cc_ps[:],
                        op0=ALU.mult, op1=ALU.mult,
                    )
                    nc.vector.tensor_tensor(
                        out=cc_ps[:], in0=e_t[:], in1=corr_ps[:], op=ALU.add
                    )
                    so = out_s[:, t * F:(t + 1) * F]
                    nc.scalar.activation(
                        out=so, in_=cc_ps[:], func=AF.Identity,
                        bias=beta_c[:], scale=iv_col[:],
                    )
                    sqt = sqpool.tile([128, F], bf16, tag="sqt")
                    nc.scalar.activation(
                        out=sqt[:], in_=so, func=AF.Square, bias=zero_c[:]
                    )
                    sq_tiles[t] = sqt
                    if t >= TSUM_LAG:
                        emit_tsum(t - TSUM_LAG)
                    # chunked stores of finished q-planes (spatial channels)
                    if t % 8 == 7:
                        q = t // 8
                        sl = slice(q * HWP, (q + 1) * HWP)
                        nc.sync.dma_start(
                            out=out_d[q, 1:64, :], in_=out_s[1:64, sl]
                        )
                        nc.sync.dma_start(
                            out=out_d[4 + q, 1:64, :], in_=out_s[65:128, sl]
                        )
                for j in range(NT - TSUM_LAG, NT):
                    emit_tsum(j)

                # ---- t rows (channel 0) ----
                t_sb = singles.tile([64, F], f32, tag="t1")
                nc.scalar.activation(
                    out=t_sb[:], in_=tpsum[0:64, :], func=AF.Sqrt,
                    bias=pone_c[0:64],
                )
                for q in range(4):
                    nc.sync.dma_start(
                        out=out_d[q, 0, :], in_=t_sb[8 * q:8 * q + 8, :]
                    )
                    nc.sync.dma_start(
                        out=out_d[4 + q, 0, :], in_=t_sb[32 + 8 * q:40 + 8 * q, :]
                    )

                # ---- P1 + AR1 for next rep (ACT idle during stores/mu) ----
                if not last:
                    emit_p1(_rep + 1)
                    emit_ar1(_rep + 1)

    return nc


def make_const_inputs(gamma: np.ndarray, beta: np.ndarray) -> dict:
    import ml_dtypes

    sign = np.ones((128, 1), np.float32)
    sign[1:64] = -1.0
    sign[65:128] = -1.0
    masktop = np.zeros((128, 1), np.float32)
    masktop[0:64] = 1.0
    maskbot = np.zeros((128, 1), np.float32)
    maskbot[64:128] = 1.0
    gneg = np.zeros((128, 1), np.float32)
    gneg[1:64, 0] = -gamma
    gneg[65:128, 0] = -gamma
    gpos = np.zeros((128, 1), np.float32)
    gpos[1:64, 0] = gamma
    gpos[65:128, 0] = gamma
    beta_col = np.zeros((128, 1), np.float32)
    beta_col[1:64, 0] = beta
    beta_col[65:128, 0] = beta
    e0t = np.zeros((128, 1), np.float32)
    e0t[0] = 1.0
    e0b = np.zeros((128, 1), np.float32)
    e0b[64] = 1.0
    onesrow = np.ones((1, 128), np.float32)
    ident = np.eye(128, dtype=np.float32)
    ccrows = np.zeros((2, 128), np.float32)
    ccrows[0, 0:64] = 1.0
    ccrows[1, 64:128] = 1.0
    fold = np.zeros((128, 128), np.float32)
    for k in range(128):
        for m in range(128):
            if k % 64 == m % 64:
                fold[k, m] = 1.0
    albase = np.zeros((128, 128 * NT), np.float32)
    tones = np.zeros((128, 128 * NT), ml_dtypes.bfloat16)
    for t in range(NT):
        tones[0:64, 128 * t + t] = 1.0
        tones[64:128, 128 * t + 32 + t] = 1.0
    return {
        "sign_col": sign, "masktop_col": masktop, "maskbot_col": maskbot,
        "gneg_col": gneg, "gpos_col": gpos, "beta_col": beta_col,
        "e0t_col": e0t, "e0b_col": e0b, "ones_row": onesrow,
        "identity": ident, "fold128": fold, "cc_rows": ccrows,
        "alhsT_base": albase, "tones_all": tones,
    }


def _legalize_waits(nc):
    """Split multi-wait sync_info into standalone single-wait
    EventSemaphore instructions: the walrus codegen in this toolchain
    only encodes one sync-wait command per engine instruction."""
    n = 0
    for fn in nc.m.functions:
        for bb in fn.blocks:
            insts = bb.instructions
            i = 0
            while i < len(insts):
                ins = insts[i]
                si = getattr(ins, "sync_info", None)
                if si is not None and si.on_wait and len(si.on_wait) > 1:
                    waits = list(si.on_wait)
                    for w in waits[:-1]:
                        ev = mybir.InstEventSemaphore(
                            name=f"WSPLIT-{n}", engine=ins.engine,
                            ins=[], outs=[],
                            sync_info=mybir.SyncInfo(on_wait=[w], on_update=[]),
                        )
                        n += 1
                        insts.insert(i, ev)
                        i += 1
                    ins.sync_info = mybir.SyncInfo(
                        on_wait=[waits[-1]], on_update=list(si.on_update or [])
                    )
                i += 1
    return n


_PROGRAM = None


def _get_program():
    global _PROGRAM
    if _PROGRAM is None:
        _PROGRAM = build_program()
        _legalize_waits(_PROGRAM)
    return _PROGRAM


_RUNNER = None


def _get_runner():
    """Cached jitted SPMD executor."""
    global _RUNNER
    if _RUNNER is not None:
        return _RUNNER
    import jax
    import jax.numpy as jnp  # noqa: F401
    from jax.experimental.shard_map import shard_map
    from jax.sharding import Mesh, PartitionSpec
    from concourse import bass2jax, mybir as _mb

    nc = _get_program()
    bass2jax.install_neuronx_cc_hook()
    partition_name = (
        nc.partition_id_tensor.name if nc.partition_id_tensor else None
    )
    in_names, out_names, out_avals, zero_outs = [], [], [], []
    for alloc in nc.m.functions[0].allocations:
        if not isinstance(alloc, _mb.MemoryLocationSet):
            continue
        name = alloc.memorylocations[0].name
        if alloc.kind == "ExternalInput":
            if name != partition_name:
                in_names.append(name)
        elif alloc.kind == "ExternalOutput":
            shape = tuple(alloc.tensor_shape)
            dtype = _mb.dt.np(alloc.dtype)
            out_names.append(name)
            out_avals.append(jax.core.ShapedArray(shape, dtype))
            zero_outs.append(np.zeros(shape, dtype))
    n_params = len(in_names)
    n_outs = len(out_avals)
    all_in_names = list(in_names) + list(out_names)
    if partition_name is not None:
        all_in_names.append(partition_name)
    donate = tuple(range(n_params, n_params + n_outs))

    def _body(*args):
        operands = list(args)
        if partition_name is not None:
            operands.append(bass2jax.partition_id_tensor())
        outs = bass2jax._bass_exec_p.bind(
            *operands,
            out_avals=tuple(out_avals),
            in_names=tuple(all_in_names),
            out_names=tuple(out_names),
            lowering_input_output_aliases=(),
            sim_require_finite=True,
            sim_require_nnan=True,
            nc=nc,
        )
        return tuple(outs)

    devices = jax.devices()[:NCORES]
    mesh = Mesh(np.asarray(devices), ("core",))
    in_specs = (PartitionSpec("core"),) * (n_params + n_outs)
    out_specs = (PartitionSpec("core"),) * n_outs
    sharded = jax.jit(
        shard_map(
            _body, mesh=mesh, in_specs=in_specs, out_specs=out_specs,
            check_rep=False,
        ),
        donate_argnums=donate,
        keep_unused=True,
    )

    def run(in_maps):
        per_core = [[np.asarray(m[n]) for n in in_names] for m in in_maps]
        concat_in = [
            np.concatenate([per_core[c][i] for c in range(NCORES)], axis=0)
            for i in range(n_params)
        ]
        concat_zeros = [
            np.zeros((NCORES * z.shape[0], *z.shape[1:]), z.dtype)
            for z in zero_outs
        ]
        out_arrs = sharded(*concat_in, *concat_zeros)
        return [
            {
                name: np.asarray(out_arrs[i]).reshape(
                    NCORES, *out_avals[i].shape
                )[c]
                for i, name in enumerate(out_names)
            }
            for c in range(NCORES)
        ]

    _RUNNER = (run, sharded, in_names, out_names, out_avals, zero_outs)
    return _RUNNER


def kernel(x: np.ndarray, gamma: np.ndarray, beta: np.ndarray) -> np.ndarray:
    run = _get_runner()[0]
    consts = make_const_inputs(
        np.asarray(gamma, np.float32), np.asarray(beta, np.float32)
    )
    x = np.asarray(x, np.float32)
    in_maps = []
    for k in range(NCORES):
        shard = np.ascontiguousarray(
            x[k * PPC:(k + 1) * PPC].reshape(PPC, C, HWP)
        )
        in_maps.append({"x": shard, **consts})
    results = run(in_maps)
    out = np.empty((B, C, H, W), np.float32)
    for k in range(NCORES):
        out[k * PPC:(k + 1) * PPC] = results[k]["out"].reshape(PPC, C, H, W)
    return out


if __name__ == "__main__":
    rng = np.random.default_rng(0)
    xs = rng.standard_normal((B, C - 1, H, W), np.float32) * 0.5
    x0 = np.sqrt(1.0 + np.sum(xs * xs, axis=1, keepdims=True))
    x = np.concatenate([x0, xs], axis=1).astype(np.float32)
    gamma = 0.5 + rng.random(C - 1, dtype=np.float32)
    beta = 0.05 * rng.standard_normal(C - 1).astype(np.float32)
    out = kernel(x=x, gamma=gamma, beta=beta)
    print(out.shape, out.dtype, np.isfinite(out).all())



# revision 49
# speedup vs baseline: 1.3971x; 1.3971x over previous
"""LorentzBatchNorm2d Trainium2 kernel (8-core SPMD), v7.

Input x: [64, 64, 64, 64] (B, C, H, W) float32, gamma/beta: [63].
Sharded data-parallel over B (8 planes per core). Channels live on SBUF
partitions (top pixel-half on partitions 0-63, bottom half on 64-127);
pixels along the free axis.

v7 changes over v6:
- The per-tile Pool (SWDGE) row DMAs feeding the cc/corr matmul rhs are
  gone. The arccosh chain writes (cf|cr) interleaved into one [64, 2F]
  bf16 tile; 8 HWDGE DMAs rearrange it into a grouped rows4 layout
  [128, J*2F] bf16 where partitions {32g, 32g+1} hold tiles t==g (mod 4)
  at free offset (t//4)*2F. PE matmuls then read rhs in place at aligned
  partition bases (0/32/64/96).
- P3 works on pairs of tiles ([128, 2F] = two PSUM banks per pool tile),
  halving per-op overheads. Elementwise passes write into out_s in
  place: e = (x*g')*cc -> out_s, out_s += corr (in-place TT), optional
  affine (in-place tensor_scalar), Square -> sqt (bf16) for tsum.
- Hybrid AR2 handling: pairs u < K0 run the AR2-independent 4-pass form
  (corr excludes iv/beta; affine applies them later); pairs u >= K0 run
  a 3-pass form with gamma*iv folded into the e-pass scalar column and
  iv/beta folded into the corr' matmul lhsT (rank-3 with a ones row in
  rows4), so no affine pass is needed.
- Elementwise work is spread across DVE, Pool, and ACT; P1 chunks are
  split ACT/DVE/Pool.
"""

import sys

sys.path.insert(0, "/opt/trn_rl_repo")

import numpy as np

import concourse.bass as bass
import concourse.tile as tile
from concourse import mybir

f32 = mybir.dt.float32
f32r = mybir.dt.float32r
bf16 = mybir.dt.bfloat16
AF = mybir.ActivationFunctionType
ALU = mybir.AluOpType

B, C, H, W = 64, 64, 64, 64
EPS = 1e-5
NCORES = 8
PPC = B // NCORES          # planes (b indices) per core = 8
HWP = H * W                # pixels per plane = 4096
PIX = PPC * HWP            # pixels per core = 32768
HALF = PIX // 2            # 16384 per partition-half
F = 512                    # pixels per tile (one PSUM bank of fp32)
NT = HALF // F             # 32 tiles per core
NP = NT // 2               # 16 pairs per core
N_TOTAL = B * H * W        # 262144

TSUM_LAG = 2               # pairs of lag before emitting tsum matmuls
G = 4                      # rhs partition groups (bases 0/32/64/96)
J = NT // G                # tiles per group = 8
NP1 = 8                    # P1 chunks (Pool can't run STT or access PSUM,
CH = HALF // NP1           # so P1 splits across ACT and DVE)
P1_ACT = range(0, 6)
P1_DVE = range(6, 8)


def build_program(repeat: int = 1, collectives: bool = True):
    nc = bass.Bass(num_devices=NCORES)

    x_d = nc.declare_dram_parameter("x", [PPC, C, HWP], f32, isOutput=False)
    out_d = nc.declare_dram_parameter("out", [PPC, C, HWP], f32, isOutput=True)

    sign_d = nc.declare_dram_parameter("sign_col", [128, 1], f32, isOutput=False)
    masktop_d = nc.declare_dram_parameter("masktop_col", [128, 1], f32, isOutput=False)
    maskbot_d = nc.declare_dram_parameter("maskbot_col", [128, 1], f32, isOutput=False)
    gneg_d = nc.declare_dram_parameter("gneg_col", [128, 1], f32, isOutput=False)
    gpos_d = nc.declare_dram_parameter("gpos_col", [128, 1], f32, isOutput=False)
    beta_d = nc.declare_dram_parameter("beta_col", [128, 1], f32, isOutput=False)
    e0t_d = nc.declare_dram_parameter("e0t_col", [128, 1], f32, isOutput=False)
    e0b_d = nc.declare_dram_parameter("e0b_col", [128, 1], f32, isOutput=False)
    onesrow_d = nc.declare_dram_parameter("ones_row", [1, 128], f32, isOutput=False)
    ident_d = nc.declare_dram_parameter("identity", [128, 128], f32, isOutput=False)
    fold_d = nc.declare_dram_parameter("fold128", [128, 128], f32, isOutput=False)
    ccrows4_d = nc.declare_dram_parameter(
        "cc_rows4", [128, 128], bf16, isOutput=False
    )
    albase_d = nc.declare_dram_parameter(
        "alhsT_base", [128, 128 * NT], f32, isOutput=False
    )
    tones_d = nc.declare_dram_parameter(
        "tones_all", [128, 128 * NT], bf16, isOutput=False
    )
    rones_d = nc.declare_dram_parameter(
        "rows_ones", [4, J * 2 * F], bf16, isOutput=False
    )

    ar1_in = nc.dram_tensor("ar1_in", [128], f32)
    ar1_out = nc.dram_tensor("ar1_out", [128], f32, addr_space="Shared")
    ar2_in = nc.dram_tensor("ar2_in", [1], f32)
    ar2_out = nc.dram_tensor("ar2_out", [1], f32, addr_space="Shared")

    rg = [list(range(NCORES))]

    from contextlib import ExitStack

    with tile.TileContext(nc) as tc:
        with ExitStack() as stack:
            resident = stack.enter_context(tc.tile_pool(name="resident", bufs=1))
            singles = stack.enter_context(tc.tile_pool(name="singles", bufs=1))
            percol = stack.enter_context(tc.tile_pool(name="percol", bufs=2))
            sqpool = stack.enter_context(tc.tile_pool(name="sqpool", bufs=4))
            psA = stack.enter_context(tc.tile_pool(name="psA", bufs=3, space="PSUM"))
            psT = stack.enter_context(tc.tile_pool(name="psT", bufs=1, space="PSUM"))
            psS = stack.enter_context(tc.tile_pool(name="psS", bufs=1, space="PSUM"))

            x_sb = resident.tile([128, HALF], f32)
            out_s = resident.tile([128, HALF], f32)
            alhsT = resident.tile([128, 128 * NT], f32)
            tones = resident.tile([128, 128 * NT], bf16)
            sink = resident.tile([128, CH], f32)
            # grouped rhs rows: partitions {32g, 32g+1} hold (cf|cr) pairs
            # for tiles t == g (mod 4) at free offset (t//4)*2F; partition
            # 32g+2 is all-ones (for the rank-3 corr' used by 3-pass pairs)
            rows4 = resident.tile([128, J * 2 * F], bf16)

            sign_c = singles.tile([128, 1], f32)
            masktop_c = singles.tile([128, 1], f32)
            maskbot_c = singles.tile([128, 1], f32)
            gneg_c = singles.tile([128, 1], f32)
            gpos_c = singles.tile([128, 1], f32)
            beta_c = singles.tile([128, 1], f32)
            e0t_c = singles.tile([128, 1], f32)
            e0b_c = singles.tile([128, 1], f32)
            onesrow_c = singles.tile([1, 128], f32)
            ident_c = singles.tile([128, 128], f32)
            fold_c = singles.tile([128, 128], f32)
            ccrows4_c = singles.tile([128, 128], bf16)
            for dst, src in (
                (sign_c, sign_d), (masktop_c, masktop_d), (maskbot_c, maskbot_d),
                (gneg_c, gneg_d), (gpos_c, gpos_d), (beta_c, beta_d),
                (e0t_c, e0t_d), (e0b_c, e0b_d), (onesrow_c, onesrow_d),
                (ident_c, ident_d), (fold_c, fold_d), (ccrows4_c, ccrows4_d),
            ):
                nc.sync.dma_start(out=dst[:], in_=src[:])
            nc.sync.dma_start(
                out=alhsT[:].bitcast(f32r), in_=albase_d[:].bitcast(f32r)
            )
            nc.sync.dma_start(out=tones[:], in_=tones_d[:])
            nc.sync.dma_start(out=rows4[2:128:32, :], in_=rones_d[:])

            zero_c = singles.tile([128, 1], f32)
            nc.vector.memset(zero_c[:], 0.0)
            neg1_c = singles.tile([128, 1], f32)
            nc.vector.memset(neg1_c[:], -1.0)
            pone_c = singles.tile([128, 1], f32)
            nc.vector.memset(pone_c[:], 1.0)

            # PE primers: absorb const-DMA waits one semaphore at a time
            psS_t = psS.tile([128, 128], f32, tag="small")
            prime_n = [0]

            def pe_prime(col_ap):
                j = prime_n[0]
                prime_n[0] += 1
                nc.tensor.matmul(
                    out=psS_t[0:1, j:j + 1], lhsT=col_ap, rhs=col_ap
                )

            pe_prime(ident_c[0:128, 0:1])
            pe_prime(fold_c[0:128, 0:1])
            pe_prime(alhsT[0:128, 0:1])
            pe_prime(onesrow_c[0:1, 0:1])

            # ---- load x (4 paired-plane DMAs) ----
            for q in range(4):
                nc.sync.dma_start(
                    out=x_sb[:, q * HWP:(q + 1) * HWP].bitcast(f32r),
                    in_=x_d[q:PPC:4].bitcast(f32r),
                )
            pe_prime(x_sb[0:128, 0:1])
            pe_prime(tones[0:128, 0:2].bitcast(f32))

            def emit_p1(r):
                """Channel sums for rep r: chunks split ACT/DVE/Pool."""
                pcol = percol.tile([128, NP1], f32, tag="pcol")
                for cth in range(NP1):
                    xin = x_sb[:, cth * CH:(cth + 1) * CH]
                    acc = pcol[:, cth:cth + 1]
                    if cth in P1_ACT:
                        nc.scalar.activation(
                            out=sink[:], in_=xin, func=AF.Copy, accum_out=acc
                        )
                    else:
                        nc.vector.reduce_sum(
                            out=acc, in_=xin, axis=mybir.AxisListType.X
                        )
                ssum = percol.tile([128, 1], f32, tag="ssum")
                nc.vector.reduce_sum(
                    out=ssum[:], in_=pcol[:], axis=mybir.AxisListType.X
                )
                nc.sync.dma_start(out=ar1_in[:], in_=ssum[:])

            def emit_ar1(r):
                if collectives:
                    nc.gpsimd.collective_compute(
                        "AllReduce", ALU.add, replica_groups=rg,
                        ins=[ar1_in[:]], outs=[ar1_out[:]],
                    )
                else:
                    nc.gpsimd.dma_start(out=ar1_out[:], in_=ar1_in[:])

            # prologue: P1 + AR1 for rep 0
            emit_p1(0)
            emit_ar1(0)

            def small_mm(lhsT, rhs, shape, tag="small"):
                ps = psS_t[0:shape[0], 0:shape[1]]
                nc.tensor.matmul(out=ps, lhsT=lhsT, rhs=rhs)
                sb = singles.tile(shape, f32, tag=f"sb_{tag}")
                nc.scalar.copy(out=sb[:], in_=ps)
                return sb

            def pe_bcast(v11, tag):
                """Broadcast a [1,1] partition-0 scalar to a [128,1] col."""
                ps = psS_t[:, 0:1]
                nc.tensor.matmul(out=ps, lhsT=onesrow_c[:], rhs=v11[:])
                col = singles.tile([128, 1], f32, tag=f"bc_{tag}")
                nc.scalar.copy(out=col[:], in_=ps)
                return col

            for _rep in range(repeat):
                last = _rep == repeat - 1

                # ---- mu chain (needs AR1 of this rep) ----
                Sg = singles.tile([128, 1], f32, tag="Sg")
                nc.sync.dma_start(out=Sg[:], in_=ar1_out[:])
                Scol = small_mm(fold_c[:], Sg[:], [128, 1], tag="fold")
                ss = singles.tile([128, 1], f32, tag="ss")
                nc.vector.scalar_tensor_tensor(
                    out=ss[:], in0=Scol[:], scalar=sign_c[:], in1=Scol[:],
                    op0=ALU.mult, op1=ALU.mult,
                )
                qsb = small_mm(ss[:], pone_c[:], [1, 1], tag="qmm")
                usb = singles.tile([1, 1], f32, tag="usb")
                nc.vector.tensor_scalar(
                    out=usb[:], in0=qsb[:], scalar1=0.5,
                    scalar2=EPS * float(N_TOTAL) ** 2,
                    op0=ALU.mult, op1=ALU.max,
                )
                su11 = singles.tile([1, 1], f32, tag="su11")
                nc.scalar.activation(
                    out=su11[:], in_=usb[:], func=AF.Sqrt, bias=zero_c[0:1]
                )
                rs11 = singles.tile([1, 1], f32, tag="rs11")
                nc.vector.reciprocal(out=rs11[:], in_=su11[:])
                rs_col = pe_bcast(rs11, "rs")
                mu_col = singles.tile([128, 1], f32, tag="mu")
                nc.vector.tensor_mul(out=mu_col[:], in0=Scol[:], in1=rs_col[:])
                mus_col = singles.tile([128, 1], f32, tag="mus")
                nc.vector.tensor_mul(out=mus_col[:], in0=mu_col[:], in1=sign_c[:])
                # masked mu columns and alpha+x0 columns
                mut = singles.tile([128, 1], f32, tag="mut")
                nc.vector.tensor_mul(out=mut[:], in0=mus_col[:], in1=masktop_c[:])
                mub = singles.tile([128, 1], f32, tag="mub")
                nc.vector.tensor_mul(out=mub[:], in0=mus_col[:], in1=maskbot_c[:])
                aat = singles.tile([128, 1], f32, tag="aat")
                nc.vector.tensor_add(out=aat[:], in0=mut[:], in1=e0t_c[:])
                aab = singles.tile([128, 1], f32, tag="aab")
                nc.vector.tensor_add(out=aab[:], in0=mub[:], in1=e0b_c[:])
                # scatter into per-tile weight blocks: col(t) = 130t + off
                al_ap = alhsT[:]
                for off, src_c in ((0, mut), (1, mub), (64, aat), (65, aab)):
                    dst = bass.AP(
                        tensor=al_ap.tensor, offset=al_ap.offset + off,
                        ap=[[128 * NT, 128], [130, NT], [1, 1]],
                    )
                    sap = src_c[:]
                    srcb = bass.AP(
                        tensor=sap.tensor, offset=sap.offset,
                        ap=[[1, 128], [0, NT], [1, 1]],
                    )
                    nc.vector.tensor_copy(out=dst.bitcast(f32r), in_=srcb)
                # w0n = -gamma * mu_s * 1/(1+mu0)
                i11 = singles.tile([1, 1], f32, tag="i11")
                nc.vector.tensor_scalar_add(
                    out=i11[:], in0=mu_col[0:1, 0:1], scalar1=1.0
                )
                inv11 = singles.tile([1, 1], f32, tag="inv11")
                nc.vector.reciprocal(out=inv11[:], in_=i11[:])
                inv_col = pe_bcast(inv11, "inv")
                w0a = singles.tile([128, 1], f32, tag="w0a")
                nc.vector.tensor_mul(out=w0a[:], in0=gneg_c[:], in1=mu_col[:])
                w0n = singles.tile([128, 1], f32, tag="w0n")
                nc.vector.tensor_mul(out=w0n[:], in0=w0a[:], in1=inv_col[:])
                # w2col: w0n masked split -> lhsCR4 rows (4 groups)
                w2col = singles.tile([128, 2], f32, tag="w2col")
                nc.vector.tensor_mul(
                    out=w2col[:, 0:1], in0=w0n[:], in1=masktop_c[:]
                )
                nc.vector.tensor_mul(
                    out=w2col[:, 1:2], in0=w0n[:], in1=maskbot_c[:]
                )
                # ---- P2: alpha / alpha+x0 matmuls into one PSUM bank ----
                apsum = psT.tile([128, F], f32, tag="acc")
                for t in range(NT):
                    nc.tensor.matmul(
                        out=apsum[:],
                        lhsT=alhsT[:, 128 * t:128 * (t + 1)].bitcast(f32r),
                        rhs=x_sb[:, t * F:(t + 1) * F].bitcast(f32r),
                        start=(t == 0), stop=(t == NT - 1),
                        skip_group_check=True,
                    )
                abank = singles.tile([128, F], f32, tag="abank")
                nc.scalar.copy(out=abank[:], in_=apsum[:])

                # ---- batched per-pixel chain on [64, F] ----
                aash = singles.tile([64, F], f32, tag="aash")
                nc.sync.dma_start(out=aash[:], in_=abank[64:128, :])
                cb = singles.tile([64, F], f32, tag="cb")
                nc.vector.tensor_scalar_max(
                    out=cb[:], in0=abank[0:64, :], scalar1=1.0 + EPS
                )
                q2 = singles.tile([64, F], f32, tag="q2")
                nc.scalar.activation(
                    out=q2[:], in_=cb[:], func=AF.Square, bias=zero_c[0:64]
                )
                sq = singles.tile([64, F], f32, tag="sq")
                nc.scalar.activation(
                    out=sq[:], in_=q2[:], func=AF.Sqrt, bias=neg1_c[0:64]
                )
                rsq = singles.tile([64, F], f32, tag="rsq")
                nc.vector.reciprocal(out=rsq[:], in_=sq[:])
                t1 = singles.tile([64, F], f32, tag="q2")  # reuse q2 buffer
                nc.vector.tensor_add(out=t1[:], in0=cb[:], in1=sq[:])
                dd = singles.tile([64, F], f32, tag="cb")  # reuse cb buffer
                nc.scalar.activation(
                    out=dd[:], in_=t1[:], func=AF.Ln, bias=zero_c[0:64]
                )
                d2col = singles.tile([64, 1], f32, tag="d2col")
                nc.vector.scalar_tensor_tensor(
                    out=sink[0:64, 0:F], in0=dd[:], scalar=1.0, in1=dd[:],
                    op0=ALU.mult, op1=ALU.mult, accum_out=d2col[:],
                )
                dsq_sb = small_mm(d2col[:], pone_c[0:64, :], [1, 1], tag="dsq")
                cfcr = singles.tile([64, 2 * F], bf16, tag="cfcr")
                nc.vector.tensor_tensor(
                    out=cfcr[:, 0:F], in0=dd[:], in1=rsq[:], op=ALU.mult,
                )
                nc.vector.tensor_tensor(
                    out=cfcr[:, F:2 * F], in0=cfcr[:, 0:F], in1=aash[:],
                    op=ALU.mult,
                )

                # ---- AR2 launch ----
                nc.sync.dma_start(out=ar2_in[:], in_=dsq_sb[:])
                if collectives:
                    nc.gpsimd.collective_compute(
                        "AllReduce", ALU.add, replica_groups=rg,
                        ins=[ar2_in[:]], outs=[ar2_out[:]],
                    )
                else:
                    nc.gpsimd.dma_start(out=ar2_out[:], in_=ar2_in[:])

                # ---- rearrange (cf|cr) rows into grouped rows4 layout ----
                # src partitions {2g+h+8j: j=0..7} x 2F elems  ->
                # dst partition 32g+h, 8 blocks of 2F contiguous
                for g in range(G):
                    for h in range(2):
                        src = cfcr[2 * g + h:64:8, :]
                        dstp = 32 * g + h
                        dst = rows4[dstp:dstp + 1, :]
                        eng = (nc.sync, nc.scalar, nc.scalar, nc.sync)[g]
                        eng.dma_start(out=dst, in_=src)

                # ---- AR2 result -> invsd / scalars ----
                vg = singles.tile([1, 1], f32, tag="vg")
                nc.sync.dma_start(out=vg[:], in_=ar2_out[:])
                sv11 = singles.tile([1, 1], f32, tag="sv11")
                nc.scalar.activation(
                    out=sv11[:], in_=vg[:], func=AF.Sqrt, bias=zero_c[0:1],
                    scale=1.0 / float(N_TOTAL),
                )
                nc.vector.tensor_scalar_add(out=sv11[:], in0=sv11[:], scalar1=EPS)
                iv11 = singles.tile([1, 1], f32, tag="iv11")
                nc.vector.reciprocal(out=iv11[:], in_=sv11[:])
                iv_col = pe_bcast(iv11, "iv")
                # lhsCR4B: rows {32g,32g+1} = iv*w0n, row 32g+2 = beta
                w3col = singles.tile([128, 3], f32, tag="w3col")
                nc.vector.tensor_scalar(
                    out=w3col[:, 0:2], in0=w2col[:], scalar1=iv_col[:],
                    scalar2=None, op0=ALU.mult,
                )
                nc.vector.tensor_copy(out=w3col[:, 2:3], in_=beta_c[:])
                w3_ps = psS_t[0:3, :]
                nc.tensor.transpose(out=w3_ps, in_=w3col[:], identity=ident_c[:])
                lhsCR4B = singles.tile([128, 128], bf16, tag="lhsCR4B")
                for g in range(G):
                    nc.vector.tensor_copy(
                        out=lhsCR4B[32 * g:32 * g + 3, :], in_=w3_ps
                    )
                # iv-scaled identity for the PE e-add (f32r-rounded)
                ividn = singles.tile([128, 128], f32, tag="ividn")
                nc.vector.tensor_scalar(
                    out=ividn[:].bitcast(f32r), in0=ident_c[:],
                    scalar1=iv_col[:], scalar2=None, op0=ALU.mult,
                )

                # ---- P1 + AR1 for next rep (only reads x_sb; collective
                # completes while P3 runs) ----
                if not last:
                    emit_p1(_rep + 1)
                    emit_ar1(_rep + 1)

                # ---- P3: 16 pairs of tiles ----
                tpsum = psT.tile([128, F], f32, tag="acc")
                sq_tiles = [None] * NP

                def emit_tsum(u):
                    for k in range(2):
                        t = 2 * u + k
                        nc.tensor.matmul(
                            out=tpsum[:],
                            lhsT=tones[:, 128 * t:128 * (t + 1)],
                            rhs=sq_tiles[u][:, k * F:(k + 1) * F],
                            start=(t == 0), stop=(t == NT - 1),
                            skip_group_check=True,
                        )

                for u in range(NP):
                    t0 = 2 * u
                    xs = x_sb[:, t0 * F:(t0 + 2) * F]
                    os = out_s[:, t0 * F:(t0 + 2) * F]
                    pairA = psA.tile([128, 2 * F], f32, tag="cc")
                    pairB = psA.tile([128, 2 * F], f32, tag="cc")
                    for k in range(2):
                        t = t0 + k
                        g, j = t % 4, t // 4
                        cf_rhs = rows4[32 * g:32 * g + 2,
                                       j * 2 * F:j * 2 * F + F]
                        nc.tensor.matmul(
                            out=pairA[:, k * F:(k + 1) * F],
                            lhsT=ccrows4_c[32 * g:32 * g + 2, :],
                            rhs=cf_rhs,
                            tile_position=(32 * g, 0),
                        )
                        # corr' = iv*w0n (x) cr + beta_bcast (rank 3)
                        cr_rhs = rows4[32 * g:32 * g + 3,
                                       j * 2 * F + F:j * 2 * F + 2 * F]
                        nc.tensor.matmul(
                            out=pairB[:, k * F:(k + 1) * F],
                            lhsT=lhsCR4B[32 * g:32 * g + 3, :],
                            rhs=cr_rhs,
                            tile_position=(32 * g, 0),
                            start=True, stop=False,
                            skip_group_check=True,
                        )
                    # e pass: os = (x * gamma) * cc  (os reused as scratch;
                    # f32r-rounded so the PE e-add can consume it)
                    nc.vector.scalar_tensor_tensor(
                        out=os.bitcast(f32r), in0=xs, scalar=gpos_c[:],
                        in1=pairA[:], op0=ALU.mult, op1=ALU.mult,
                    )
                    # PE e-add: pairB += iv * e  (so = iv*(e+corr)+beta)
                    for k in range(2):
                        nc.tensor.matmul(
                            out=pairB[:, k * F:(k + 1) * F],
                            lhsT=ividn[:].bitcast(f32r),
                            rhs=out_s[:, (t0 + k) * F:(t0 + k + 1) * F
                                      ].bitcast(f32r),
                            start=False, stop=True,
                            skip_group_check=True,
                        )
                    # copy so -> out_s (for stores); Square -> sqt (for t)
                    # (f32r-rounded: the verifier requires all out_s writers
                    # to round since the PE e-add consumes out_s as f32r)
                    if u % 2 == 0 and u < 12:
                        nc.scalar.copy(out=os.bitcast(f32r), in_=pairB[:])
                    else:
                        nc.vector.tensor_copy(out=os.bitcast(f32r), in_=pairB[:])
                    sqt = sqpool.tile([128, 2 * F], bf16, tag="sqt")
                    nc.scalar.activation(
                        out=sqt[:], in_=pairB[:], func=AF.Square,
                        bias=zero_c[:],
                    )
                    sq_tiles[u] = sqt
                    if u >= TSUM_LAG:
                        emit_tsum(u - TSUM_LAG)
                    # chunked stores of finished q-planes (spatial channels)
                    if u % 4 == 3:
                        q = u // 4
                        sl = slice(q * HWP, (q + 1) * HWP)
                        nc.sync.dma_start(
                            out=out_d[q, 1:64, :], in_=out_s[1:64, sl]
                        )
                        nc.sync.dma_start(
                            out=out_d[4 + q, 1:64, :], in_=out_s[65:128, sl]
                        )
                for u in range(NP - TSUM_LAG, NP):
                    emit_tsum(u)

                # ---- t rows (channel 0) ----
                t_sb = singles.tile([64, F], f32, tag="t_sb")
                nc.scalar.activation(
                    out=t_sb[:], in_=tpsum[0:64, :], func=AF.Sqrt,
                    bias=pone_c[0:64],
                )
                nc.sync.dma_start(out=out_d[0:4, 0, :], in_=t_sb[0:32, :])
                nc.sync.dma_start(out=out_d[4:8, 0, :], in_=t_sb[32:64, :])

    return nc


def make_const_inputs(gamma: np.ndarray, beta: np.ndarray) -> dict:
    import ml_dtypes

    sign = np.ones((128, 1), np.float32)
    sign[1:64] = -1.0
    sign[65:128] = -1.0
    masktop = np.zeros((128, 1), np.float32)
    masktop[0:64] = 1.0
    maskbot = np.zeros((128, 1), np.float32)
    maskbot[64:128] = 1.0
    gneg = np.zeros((128, 1), np.float32)
    gneg[1:64, 0] = -gamma
    gneg[65:128, 0] = -gamma
    gpos = np.zeros((128, 1), np.float32)
    gpos[1:64, 0] = gamma
    gpos[65:128, 0] = gamma
    beta_col = np.zeros((128, 1), np.float32)
    beta_col[1:64, 0] = beta
    beta_col[65:128, 0] = beta
    e0t = np.zeros((128, 1), np.float32)
    e0t[0] = 1.0
    e0b = np.zeros((128, 1), np.float32)
    e0b[64] = 1.0
    onesrow = np.ones((1, 128), np.float32)
    ident = np.eye(128, dtype=np.float32)
    ccrows4 = np.zeros((128, 128), ml_dtypes.bfloat16)
    for g in range(4):
        ccrows4[32 * g + 0, 0:64] = 1.0
        ccrows4[32 * g + 1, 64:128] = 1.0
    fold = np.zeros((128, 128), np.float32)
    for k in range(128):
        for m in range(128):
            if k % 64 == m % 64:
                fold[k, m] = 1.0
    albase = np.zeros((128, 128 * NT), np.float32)
    tones = np.zeros((128, 128 * NT), ml_dtypes.bfloat16)
    for t in range(NT):
        tones[0:64, 128 * t + t] = 1.0
        tones[64:128, 128 * t + 32 + t] = 1.0
    return {
        "sign_col": sign, "masktop_col": masktop, "maskbot_col": maskbot,
        "gneg_col": gneg, "gpos_col": gpos, "beta_col": beta_col,
        "e0t_col": e0t, "e0b_col": e0b, "ones_row": onesrow,
        "identity": ident, "fold128": fold, "cc_rows4": ccrows4,
        "alhsT_base": albase, "tones_all": tones,
        "rows_ones": np.ones((4, J * 2 * F), ml_dtypes.bfloat16),
    }


def _legalize_waits(nc):
    """Split multi-wait sync_info into standalone single-wait
    EventSemaphore instructions: the walrus codegen in this toolchain
    only encodes one sync-wait command per engine instruction."""
    n = 0
    for fn in nc.m.functions:
        for bb in fn.blocks:
            insts = bb.instructions
            i = 0
            while i < len(insts):
                ins = insts[i]
                si = getattr(ins, "sync_info", None)
                if si is not None and si.on_wait and len(si.on_wait) > 1:
                    waits = list(si.on_wait)
                    for w in waits[:-1]:
                        ev = mybir.InstEventSemaphore(
                            name=f"WSPLIT-{n}", engine=ins.engine,
                            ins=[], outs=[],
                            sync_info=mybir.SyncInfo(on_wait=[w], on_update=[]),
                        )
                        n += 1
                        insts.insert(i, ev)
                        i += 1
                    ins.sync_info = mybir.SyncInfo(
                        on_wait=[waits[-1]], on_update=list(si.on_update or [])
                    )
                i += 1
    return n


_PROGRAM = None


def _get_program():
    global _PROGRAM
    if _PROGRAM is None:
        _PROGRAM = build_program()
        _legalize_waits(_PROGRAM)
    return _PROGRAM


_RUNNER = None


def _get_runner():
    """Cached jitted SPMD executor."""
    global _RUNNER
    if _RUNNER is not None:
        return _RUNNER
    import jax
    import jax.numpy as jnp  # noqa: F401
    from jax.experimental.shard_map import shard_map
    from jax.sharding import Mesh, PartitionSpec
    from concourse import bass2jax, mybir as _mb

    nc = _get_program()
    bass2jax.install_neuronx_cc_hook()
    partition_name = (
        nc.partition_id_tensor.name if nc.partition_id_tensor else None
    )
    in_names, out_names, out_avals, zero_outs = [], [], [], []
    for alloc in nc.m.functions[0].allocations:
        if not isinstance(alloc, _mb.MemoryLocationSet):
            continue
        name = alloc.memorylocations[0].name
        if alloc.kind == "ExternalInput":
            if name != partition_name:
                in_names.append(name)
        elif alloc.kind == "ExternalOutput":
            shape = tuple(alloc.tensor_shape)
            dtype = _mb.dt.np(alloc.dtype)
            out_names.append(name)
            out_avals.append(jax.core.ShapedArray(shape, dtype))
            zero_outs.append(np.zeros(shape, dtype))
    n_params = len(in_names)
    n_outs = len(out_avals)
    all_in_names = list(in_names) + list(out_names)
    if partition_name is not None:
        all_in_names.append(partition_name)
    donate = tuple(range(n_params, n_params + n_outs))

    def _body(*args):
        operands = list(args)
        if partition_name is not None:
            operands.append(bass2jax.partition_id_tensor())
        outs = bass2jax._bass_exec_p.bind(
            *operands,
            out_avals=tuple(out_avals),
            in_names=tuple(all_in_names),
            out_names=tuple(out_names),
            lowering_input_output_aliases=(),
            sim_require_finite=True,
            sim_require_nnan=True,
            nc=nc,
        )
        return tuple(outs)

    devices = jax.devices()[:NCORES]
    mesh = Mesh(np.asarray(devices), ("core",))
    in_specs = (PartitionSpec("core"),) * (n_params + n_outs)
    out_specs = (PartitionSpec("core"),) * n_outs
    sharded = jax.jit(
        shard_map(
            _body, mesh=mesh, in_specs=in_specs, out_specs=out_specs,
            check_rep=False,
        ),
        donate_argnums=donate,
        keep_unused=True,
    )

    def run(in_maps):
        per_core = [[np.asarray(m[n]) for n in in_names] for m in in_maps]
        concat_in = [
            np.concatenate([per_core[c][i] for c in range(NCORES)], axis=0)
            for i in range(n_params)
        ]
        concat_zeros = [
            np.zeros((NCORES * z.shape[0], *z.shape[1:]), z.dtype)
            for z in zero_outs
        ]
        out_arrs = sharded(*concat_in, *concat_zeros)
        return [
            {
                name: np.asarray(out_arrs[i]).reshape(
                    NCORES, *out_avals[i].shape
                )[c]
                for i, name in enumerate(out_names)
            }
            for c in range(NCORES)
        ]

    _RUNNER = (run, sharded, in_names, out_names, out_avals, zero_outs)
    return _RUNNER


def kernel(x: np.ndarray, gamma: np.ndarray, beta: np.ndarray) -> np.ndarray:
    run = _get_runner()[0]
    consts = make_const_inputs(
        np.asarray(gamma, np.float32), np.asarray(beta, np.float32)
    )
    x = np.asarray(x, np.float32)
    in_maps = []
    for k in range(NCORES):
        shard = np.ascontiguousarray(
            x[k * PPC:(k + 1) * PPC].reshape(PPC, C, HWP)
        )
        in_maps.append({"x": shard, **consts})
    results = run(in_maps)
    out = np.empty((B, C, H, W), np.float32)
    for k in range(NCORES):
        out[k * PPC:(k + 1) * PPC] = results[k]["out"].reshape(PPC, C, H, W)
    return out


if __name__ == "__main__":
    rng = np.random.default_rng(0)
    xs = rng.standard_normal((B, C - 1, H, W), np.float32) * 0.5
    x0 = np.sqrt(1.0 + np.sum(xs * xs, axis=1, keepdims=True))
    x = np.concatenate([x0, xs], axis=1).astype(np.float32)
    gamma = 0.5 + rng.random(C - 1, dtype=np.float32)
    beta = 0.05 * rng.standard_normal(C - 1).astype(np.float32)
    out = kernel(x=x, gamma=gamma, beta=beta)
    print(out.shape, out.dtype, np.isfinite(out).all())
